# revision 26
# baseline (speedup 1.0000x reference)
"""GAE (generalized advantage estimation) kernel for trn2, 8 NeuronCores.

Computes advantages[t] = delta[t] + gl * advantages[t+1] (reverse scan over
T-1=1023 steps) for deltas = rewards[:-1] + gamma*values[1:] - values[:-1],
for 32768 independent batch columns, data-parallel over 8 cores.

Per core (R, V in [1024, 4096] -> A [1023, 4096], all bf16 on device; host
converts f32 <-> bf16 at the boundary):
    out_m = L1 @ R_m + L3 @ V_m          (two 128x128 bf16 matmuls per
                                          128-row time block, f32 PSUM)
where L1[i,j] = gl^(j-i) (j>=i) covers the reward terms and L3 combines the
-v_t and +gamma*v_{t+1} coefficients (phi_0 = -1, phi_d = gl^(d-1)(gamma-gl)).
The cross-block tail is carried by H_m = adv_{128m} + (gamma/gl) v_{128m},
folded into row 0 of the next block's V tile (whose natural L3 row-0
coefficient column is replaced by gl^(128-i)).  Carry chain m = 7 -> 0:
    save:  v0s_m   = kappa_m * V_m[0]          (DVE, before row 0 is poked)
    poke:  V_{m-1}[0] = v0s_m + stage_m[0]     (GpSimd adds, per 1024-col ch)
with kappa_7 = gamma/gl (block 7's psum row 0 is exact) and
kappa_{m<7} = gamma/gl - 1 (psum row 0 = adv + v0 because the -1 diag entry
of L3 row 0 was displaced by the carry column).  The displaced -v0 on output
rows 0, 128, ..., 768 is repaired on the host during unshard (7 rows).

All I/O is bf16 (halves DMA bytes); PSUM accumulates in f32.  DMAs are
spread over the three DGE queues (sync / scalar / gpsimd); PSUM->SBUF
stages run on Act + DVE per 1024-col chunk so the carry chain pipelines.
"""
import numpy as np

GAMMA = 0.99
LAM = 0.95
GL = GAMMA * LAM
T = 1024
B = 32768
NCORES = 8
BC = B // NCORES          # 4096 batch cols per core
P = 128                   # partitions / time-block size
NB = T // P               # 8 time blocks
CH = 1024                 # carry-chain chunk width (4 chunks per core)
NCHN = BC // CH           # 4 chains
NW = 512                  # matmul moving width (1 PSUM bank, fp32)


def _make_consts():
    ii = np.arange(P)[None, :]  # out row i
    jj = np.arange(P)[:, None]  # data row j  (lhsT layout [j, i])
    L1 = np.where(jj >= ii, GL ** (jj - ii), 0.0)
    L1z = L1.copy()
    L1z[P - 1, :] = 0.0                      # block 7: no r_1023
    D = jj - ii
    L3 = np.where(D > 0, GL ** np.clip(D - 1, 0, None) * (GAMMA - GL), 0.0)
    L3 = L3 + np.where(D == 0, -1.0, 0.0)
    L3c = L3.copy()
    L3c[0, :] = GL ** (P - np.arange(P))     # carry coefficient column
    L3z = L3.copy()
    L3z[P - 1, :] = GAMMA * GL ** (126.0 - np.arange(P))  # v_1023 gamma-only
    import ml_dtypes
    bf = ml_dtypes.bfloat16
    return (L1.astype(bf), L1z.astype(bf), L3c.astype(bf), L3z.astype(bf))


def _build():
    import concourse.bacc as bacc
    import concourse.mybir as mybir
    from concourse.tile import TileContext

    f32 = mybir.dt.float32
    bf16 = mybir.dt.bfloat16
    add = mybir.AluOpType.add

    nc = bacc.Bacc("TRN2")
    R = nc.dram_tensor("R", [T, BC], bf16, kind="ExternalInput")
    V = nc.dram_tensor("V", [T, BC], bf16, kind="ExternalInput")
    L1 = nc.dram_tensor("L1", [P, P], bf16, kind="ExternalInput")
    L1z = nc.dram_tensor("L1z", [P, P], bf16, kind="ExternalInput")
    L3c = nc.dram_tensor("L3c", [P, P], bf16, kind="ExternalInput")
    L3z = nc.dram_tensor("L3z", [P, P], bf16, kind="ExternalInput")
    A = nc.dram_tensor("A", [T - 1, BC], bf16, kind="ExternalOutput")

    # DMA cost is charged to the issuing engine, so balance engine time:
    # SP does nothing else -> most DMA units; Act carries the psum->stage
    # copies -> few units; Pool does DMAs only; DVE (no DMA capability) does
    # saves, pokes, and the copy overflow.  DMAs are [128, 1024] units
    # emitted just-in-time so each engine queue runs in need order.
    dma_pat = (["sync", "gpsimd"] * 5 + ["scalar"]
               + ["sync", "gpsimd"] * 5 + ["sync", "scalar", "gpsimd"])
    # psum->stage copy engine per (m, ch): mostly Act (it carries few
    # DMAs), a few on DVE; block 0 (the tail) splits so copies run
    # concurrently.
    def copy_eng(m, ch):
        if m == 0:
            return "vector" if ch in (0, 2) else "scalar"
        t = (NB - 1 - m) * NCHN + ch
        return "vector" if t % 8 == 5 else "scalar"

    with TileContext(nc) as tc:
        with (
            tc.tile_pool(name="cst", bufs=1) as cst,
            tc.tile_pool(name="rp", bufs=4) as rp,
            tc.tile_pool(name="vp", bufs=8) as vp,
            tc.tile_pool(name="stp", bufs=4) as stp,
            tc.tile_pool(name="v0p", bufs=3) as v0p,
            tc.tile_pool(name="ps", bufs=4, space="PSUM") as ps,
        ):
            l1 = cst.tile([P, P], bf16, tag="l1")
            l1z = cst.tile([P, P], bf16, tag="l1z")
            l3c = cst.tile([P, P], bf16, tag="l3c")
            l3z = cst.tile([P, P], bf16, tag="l3z")
            # block-7 matrices first (needed soonest); keep the heavy copy
            # engine (Act) free of const traffic.
            nc.scalar.dma_start(out=l1z[:, :], in_=L1z[:, :])
            nc.scalar.dma_start(out=l3z[:, :], in_=L3z[:, :])
            nc.sync.dma_start(out=l1[:, :], in_=L1[:, :])
            nc.gpsimd.dma_start(out=l3c[:, :], in_=L3c[:, :])

            rt, vt = {}, {}
            dma_n = [0]

            def next_q():
                q = dma_pat[dma_n[0] % len(dma_pat)]
                dma_n[0] += 1
                return getattr(nc, q)

            def emit_loads(m):
                r = rp.tile([P, BC], bf16, tag="r")
                v = vp.tile([P, BC], bf16, tag="v")
                for ch in range(NCHN):
                    cs = slice(ch * CH, (ch + 1) * CH)
                    rows = slice(m * P, (m + 1) * P)
                    next_q().dma_start(out=r[:, cs], in_=R[rows, cs])
                    next_q().dma_start(out=v[:, cs], in_=V[rows, cs])
                rt[m] = r
                vt[m] = v

            emit_loads(NB - 1)
            emit_loads(NB - 2)

            v0s = {}

            def save(m):
                kap = (GAMMA / GL) if m == NB - 1 else (GAMMA / GL - 1.0)
                s = v0p.tile([1, BC], bf16, tag="v0")
                nc.vector.tensor_scalar_mul(s[0:1, :], vt[m][0:1, :], kap)
                v0s[m] = s

            save(NB - 1)
            for m in range(NB - 1, -1, -1):
                if m >= 2:
                    emit_loads(m - 2)
                if m >= 1:
                    save(m - 1)
                lhs1 = l1z if m == NB - 1 else l1
                lhs3 = l3z if m == NB - 1 else l3c
                stage = stp.tile([P, BC], bf16, tag="st")
                nrows = P - 1 if m == NB - 1 else P
                for ch in range(NCHN):
                    cs = slice(ch * CH, (ch + 1) * CH)
                    pt = ps.tile([P, CH], f32, tag="ps")
                    for s2 in range(CH // NW):
                        ns = slice(s2 * NW, (s2 + 1) * NW)
                        gs = slice(ch * CH + s2 * NW, ch * CH + (s2 + 1) * NW)
                        nc.tensor.matmul(pt[:, ns], lhs1[:, :], rt[m][:, gs],
                                         start=True, stop=False)
                        nc.tensor.matmul(pt[:, ns], lhs3[:, :], vt[m][:, gs],
                                         start=False, stop=True)
                    eng = copy_eng(m, ch)
                    if eng == "scalar":
                        nc.scalar.copy(stage[:, cs], pt[:, :])
                    else:
                        nc.vector.tensor_copy(stage[:, cs], pt[:, :])
                    if m >= 1:
                        # carry poke: V_{m-1}[0] = kappa*v0_m + stage_m[0]
                        nc.vector.tensor_tensor(
                            vt[m - 1][0:1, cs], v0s[m][0:1, cs],
                            stage[0:1, cs], add)
                    next_q().dma_start(
                        out=A[m * P:m * P + nrows, cs],
                        in_=stage[0:nrows, cs])
    nc.finalize()
    return nc


_NC_CACHE = None


def kernel(rewards: np.ndarray, values: np.ndarray) -> np.ndarray:
    import ml_dtypes
    from concourse.bass_utils import run_bass_kernel_spmd

    bf = ml_dtypes.bfloat16
    rewards = np.asarray(rewards)
    values = np.asarray(values)

    global _NC_CACHE
    if _NC_CACHE is None:
        _NC_CACHE = _build()
    nc = _NC_CACHE

    L1, L1z, L3c, L3z = _make_consts()
    rbf = rewards.astype(bf)
    vbf = values.astype(bf)
    in_maps = []
    for c in range(NCORES):
        cs = slice(c * BC, (c + 1) * BC)
        in_maps.append({
            "R": np.ascontiguousarray(rbf[:, cs]),
            "V": np.ascontiguousarray(vbf[:, cs]),
            "L1": L1, "L1z": L1z, "L3c": L3c, "L3z": L3z,
        })
    res = run_bass_kernel_spmd(nc, in_maps, core_ids=list(range(NCORES)))
    out = np.empty((T - 1, B), dtype=np.float32)
    for c in range(NCORES):
        out[:, c * BC:(c + 1) * BC] = np.asarray(res.results[c]["A"],
                                                 dtype=np.float32)
    # Repair the carry-displaced -v0 on output rows 0, 128, ..., 768: the
    # device stored psum row 0 = adv + v0 there (L3 row 0 held the carry
    # column instead of its -1 diagonal entry).
    vb32 = vbf.astype(np.float32)
    for m in range(NB - 1):
        out[m * P, :] -= vb32[m * P, :]
    return out


# revision 41
# speedup vs baseline: 1.0539x; 1.0539x over previous
"""GAE (generalized advantage estimation) kernel for trn2, 8 NeuronCores.

Computes advantages[t] = delta[t] + gl * advantages[t+1] (reverse scan over
T-1=1023 steps) for deltas = rewards[:-1] + gamma*values[1:] - values[:-1],
for 32768 independent batch columns, data-parallel over 8 cores.

Per core (R, V in [1024, 4096] -> A [1023, 4096], all bf16 on device; host
converts f32 <-> bf16 at the boundary):
    out_m = L1 @ R_m + L3 @ V_m          (two 128x128 bf16 matmuls per
                                          128-row time block, f32 PSUM)
where L1[i,j] = gl^(j-i) (j>=i) covers the reward terms and L3 combines the
-v_t and +gamma*v_{t+1} coefficients (phi_0 = -1, phi_d = gl^(d-1)(gamma-gl)).
The cross-block tail is carried by H_m = adv_{128m} + (gamma/gl) v_{128m},
folded into row 0 of the next block's V tile (whose natural L3 row-0
coefficient column is replaced by gl^(128-i)).  Carry chain m = 7 -> 0:
    save:  v0s_m   = kappa_m * V_m[0]          (DVE, before row 0 is poked)
    poke:  V_{m-1}[0] = v0s_m + stage_m[0]     (GpSimd adds, per 1024-col ch)
with kappa_7 = gamma/gl (block 7's psum row 0 is exact) and
kappa_{m<7} = gamma/gl - 1 (psum row 0 = adv + v0 because the -1 diag entry
of L3 row 0 was displaced by the carry column).  The displaced -v0 on output
rows 0, 128, ..., 768 is repaired on the host during unshard (7 rows).

All I/O is bf16 (halves DMA bytes); PSUM accumulates in f32.  DMAs are
spread over the three DGE queues (sync / scalar / gpsimd); PSUM->SBUF
stages run on Act + DVE per 1024-col chunk so the carry chain pipelines.
"""
import numpy as np

GAMMA = 0.99
LAM = 0.95
GL = GAMMA * LAM
T = 1024
B = 32768
NCORES = 8
BC = B // NCORES          # 4096 batch cols per core
P = 128                   # partitions / time-block size
NB = T // P               # 8 time blocks
CH = 1024                 # carry-chain chunk width (4 chunks per core)
NCHN = BC // CH           # 4 chains
NW = 512                  # matmul moving width (1 PSUM bank, fp32)


def _make_consts():
    ii = np.arange(P)[None, :]  # out row i
    jj = np.arange(P)[:, None]  # data row j  (lhsT layout [j, i])
    L1 = np.where(jj >= ii, GL ** (jj - ii), 0.0)
    L1z = L1.copy()
    L1z[P - 1, :] = 0.0                      # block 7: no r_1023
    D = jj - ii
    L3 = np.where(D > 0, GL ** np.clip(D - 1, 0, None) * (GAMMA - GL), 0.0)
    L3 = L3 + np.where(D == 0, -1.0, 0.0)
    L3c = L3.copy()
    L3c[0, :] = GL ** (P - np.arange(P))     # carry coefficient column
    L3z = L3.copy()
    L3z[P - 1, :] = GAMMA * GL ** (126.0 - np.arange(P))  # v_1023 gamma-only
    import ml_dtypes
    bf = ml_dtypes.bfloat16
    return np.concatenate([L1, L1z, L3c, L3z], axis=1).astype(bf)


def _build(copy_dve=None, pat=None):
    import concourse.bacc as bacc
    import concourse.mybir as mybir
    from concourse.tile import TileContext

    f32 = mybir.dt.float32
    bf16 = mybir.dt.bfloat16
    add = mybir.AluOpType.add

    nc = bacc.Bacc("TRN2")
    R = nc.dram_tensor("R", [T, BC], bf16, kind="ExternalInput")
    V = nc.dram_tensor("V", [T, BC], bf16, kind="ExternalInput")
    # all four 128x128 coefficient matrices packed side by side -> one DMA
    LL = nc.dram_tensor("LL", [P, 4 * P], bf16, kind="ExternalInput")
    A = nc.dram_tensor("A", [T - 1, BC], bf16, kind="ExternalOutput")

    # DMA cost is charged to the issuing engine, so balance engine time:
    # SP does nothing else -> most DMA units; Act carries the psum->stage
    # copies -> few units; Pool does DMAs only; DVE (no DMA capability) does
    # saves, pokes, and the copy overflow.  DMAs are [128, 1024] units
    # emitted just-in-time so each engine queue runs in need order.
    dma_pat = pat or (["gpsimd", "sync"] * 5 + ["scalar"]
                      + ["gpsimd", "sync"] * 5 + ["gpsimd", "scalar", "sync"])
    # psum->stage copy engine per (m, ch): mostly Act (it carries few
    # DMAs), a few on DVE; block 0 (the tail) splits so copies run
    # concurrently.
    if copy_dve is None:
        copy_dve = {2, 5, 13, 21, 28, 30}

    def copy_eng(m, ch):
        t = (NB - 1 - m) * NCHN + ch
        return "vector" if t in copy_dve else "scalar"

    with TileContext(nc) as tc:
        with (
            tc.tile_pool(name="cst", bufs=1) as cst,
            tc.tile_pool(name="rp", bufs=4) as rp,
            tc.tile_pool(name="vp", bufs=8) as vp,
            tc.tile_pool(name="stp", bufs=4) as stp,
            tc.tile_pool(name="v0p", bufs=3) as v0p,
            tc.tile_pool(name="ps", bufs=4, space="PSUM") as ps,
        ):
            ll = cst.tile([P, 4 * P], bf16, tag="ll")
            nc.scalar.dma_start(out=ll[:, :], in_=LL[:, :])
            l1 = ll[:, 0:P]
            l1z = ll[:, P:2 * P]
            l3c = ll[:, 2 * P:3 * P]
            l3z = ll[:, 3 * P:4 * P]

            # PE p-state warm-up: ~3.2us of tiny matmuls on a zero tile so
            # the tensor engine is at full clock when the first real matmul
            # issues.  They write the first psum buffer, which the first
            # real matmul resets (start=True), so no cleanup is needed.
            warm = cst.tile([P, 64], bf16, tag="warm")
            nc.gpsimd.memset(warm[:, :], 0.0)
            wpt = ps.tile([P, CH], f32, tag="ps")
            for _ in range(40):
                nc.tensor.matmul(wpt[0:64, 0:64], warm[:, :], warm[:, :],
                                 start=True, stop=True)

            rt, vt = {}, {}
            dma_n = [0]

            def next_q():
                q = dma_pat[dma_n[0] % len(dma_pat)]
                dma_n[0] += 1
                return getattr(nc, q)

            def emit_loads(m):
                r = rp.tile([P, BC], bf16, tag="r")
                v = vp.tile([P, BC], bf16, tag="v")
                for ch in range(NCHN):
                    cs = slice(ch * CH, (ch + 1) * CH)
                    rows = slice(m * P, (m + 1) * P)
                    next_q().dma_start(out=r[:, cs], in_=R[rows, cs])
                    next_q().dma_start(out=v[:, cs], in_=V[rows, cs])
                rt[m] = r
                vt[m] = v

            emit_loads(NB - 1)
            emit_loads(NB - 2)

            v0s = {}

            def save(m):
                kap = (GAMMA / GL) if m == NB - 1 else (GAMMA / GL - 1.0)
                s = v0p.tile([1, BC], bf16, tag="v0")
                nc.vector.tensor_scalar_mul(s[0:1, :], vt[m][0:1, :], kap)
                v0s[m] = s

            save(NB - 1)
            for m in range(NB - 1, -1, -1):
                if m >= 2:
                    emit_loads(m - 2)
                if m >= 1:
                    save(m - 1)
                lhs1 = l1z if m == NB - 1 else l1
                lhs3 = l3z if m == NB - 1 else l3c
                stage = stp.tile([P, BC], bf16, tag="st")
                nrows = P - 1 if m == NB - 1 else P
                for ch in range(NCHN):
                    cs = slice(ch * CH, (ch + 1) * CH)
                    pt = ps.tile([P, CH], f32, tag="ps")
                    for s2 in range(CH // NW):
                        ns = slice(s2 * NW, (s2 + 1) * NW)
                        gs = slice(ch * CH + s2 * NW, ch * CH + (s2 + 1) * NW)
                        nc.tensor.matmul(pt[:, ns], lhs1, rt[m][:, gs],
                                         start=True, stop=False)
                        nc.tensor.matmul(pt[:, ns], lhs3, vt[m][:, gs],
                                         start=False, stop=True)
                    eng = copy_eng(m, ch)
                    if eng == "scalar":
                        nc.scalar.copy(stage[:, cs], pt[:, :])
                    else:
                        nc.vector.tensor_copy(stage[:, cs], pt[:, :])
                    if m >= 1:
                        # carry poke: V_{m-1}[0] = kappa*v0_m + stage_m[0]
                        nc.vector.tensor_tensor(
                            vt[m - 1][0:1, cs], v0s[m][0:1, cs],
                            stage[0:1, cs], add)
                    next_q().dma_start(
                        out=A[m * P:m * P + nrows, cs],
                        in_=stage[0:nrows, cs])
    nc.finalize()
    return nc


_NC_CACHE = None


def kernel(rewards: np.ndarray, values: np.ndarray) -> np.ndarray:
    import ml_dtypes
    from concourse.bass_utils import run_bass_kernel_spmd

    bf = ml_dtypes.bfloat16
    rewards = np.asarray(rewards)
    values = np.asarray(values)

    global _NC_CACHE
    if _NC_CACHE is None:
        _NC_CACHE = _build()
    nc = _NC_CACHE

    LL = _make_consts()
    rbf = rewards.astype(bf)
    vbf = values.astype(bf)
    in_maps = []
    for c in range(NCORES):
        cs = slice(c * BC, (c + 1) * BC)
        in_maps.append({
            "R": np.ascontiguousarray(rbf[:, cs]),
            "V": np.ascontiguousarray(vbf[:, cs]),
            "LL": LL,
        })
    res = run_bass_kernel_spmd(nc, in_maps, core_ids=list(range(NCORES)))
    out = np.empty((T - 1, B), dtype=np.float32)
    for c in range(NCORES):
        out[:, c * BC:(c + 1) * BC] = np.asarray(res.results[c]["A"],
                                                 dtype=np.float32)
    # Repair the carry-displaced -v0 on output rows 0, 128, ..., 768: the
    # device stored psum row 0 = adv + v0 there (L3 row 0 held the carry
    # column instead of its -1 diagonal entry).
    vb32 = vbf.astype(np.float32)
    for m in range(NB - 1):
        out[m * P, :] -= vb32[m * P, :]
    return out


# revision 42
# speedup vs baseline: 1.0637x; 1.0093x over previous
"""GAE (generalized advantage estimation) kernel for trn2, 8 NeuronCores.

Computes advantages[t] = delta[t] + gl * advantages[t+1] (reverse scan over
T-1=1023 steps) for deltas = rewards[:-1] + gamma*values[1:] - values[:-1],
for 32768 independent batch columns, data-parallel over 8 cores.

Per core (R, V in [1024, 4096] -> A [1023, 4096], all bf16 on device; host
converts f32 <-> bf16 at the boundary):
    out_m = L1 @ R_m + L3 @ V_m          (two 128x128 bf16 matmuls per
                                          128-row time block, f32 PSUM)
where L1[i,j] = gl^(j-i) (j>=i) covers the reward terms and L3 combines the
-v_t and +gamma*v_{t+1} coefficients (phi_0 = -1, phi_d = gl^(d-1)(gamma-gl)).
The cross-block tail is carried by H_m = adv_{128m} + (gamma/gl) v_{128m},
folded into row 0 of the next block's V tile (whose natural L3 row-0
coefficient column is replaced by gl^(128-i)).  Carry chain m = 7 -> 0:
    save:  v0s_m   = kappa_m * V_m[0]          (DVE, before row 0 is poked)
    poke:  V_{m-1}[0] = v0s_m + stage_m[0]     (GpSimd adds, per 1024-col ch)
with kappa_7 = gamma/gl (block 7's psum row 0 is exact) and
kappa_{m<7} = gamma/gl - 1 (psum row 0 = adv + v0 because the -1 diag entry
of L3 row 0 was displaced by the carry column).  The displaced -v0 on output
rows 0, 128, ..., 768 is repaired on the host during unshard (7 rows).

All I/O is bf16 (halves DMA bytes); PSUM accumulates in f32.  Cost-model
informed schedule: DMA transfer time bills to the issuing engine, so the
[128,1024] load/store units are spread SP/Pool-heavy with a few on Act
(which carries most PSUM->SBUF stage copies); saves/pokes and a few
copies go to DVE (no DMA capability); ~40 tiny warm-up matmuls on a zero
tile bring the PE out of its low p-state before the first real matmul.
"""
import numpy as np

GAMMA = 0.99
LAM = 0.95
GL = GAMMA * LAM
T = 1024
B = 32768
NCORES = 8
BC = B // NCORES          # 4096 batch cols per core
P = 128                   # partitions / time-block size
NB = T // P               # 8 time blocks
CH = 1024                 # carry-chain chunk width (4 chunks per core)
NCHN = BC // CH           # 4 chains
NW = 512                  # matmul moving width (1 PSUM bank, fp32)


def _make_consts():
    ii = np.arange(P)[None, :]  # out row i
    jj = np.arange(P)[:, None]  # data row j  (lhsT layout [j, i])
    L1 = np.where(jj >= ii, GL ** (jj - ii), 0.0)
    L1z = L1.copy()
    L1z[P - 1, :] = 0.0                      # block 7: no r_1023
    D = jj - ii
    L3 = np.where(D > 0, GL ** np.clip(D - 1, 0, None) * (GAMMA - GL), 0.0)
    L3 = L3 + np.where(D == 0, -1.0, 0.0)
    L3c = L3.copy()
    L3c[0, :] = GL ** (P - np.arange(P))     # carry coefficient column
    L3z = L3.copy()
    L3z[P - 1, :] = GAMMA * GL ** (126.0 - np.arange(P))  # v_1023 gamma-only
    import ml_dtypes
    bf = ml_dtypes.bfloat16
    return np.concatenate([L1, L1z, L3c, L3z], axis=1).astype(bf)


def _build(copy_dve=None, pat=None):
    import concourse.bacc as bacc
    import concourse.mybir as mybir
    from concourse.tile import TileContext

    f32 = mybir.dt.float32
    bf16 = mybir.dt.bfloat16
    add = mybir.AluOpType.add

    nc = bacc.Bacc("TRN2")
    R = nc.dram_tensor("R", [T, BC], bf16, kind="ExternalInput")
    V = nc.dram_tensor("V", [T, BC], bf16, kind="ExternalInput")
    # all four 128x128 coefficient matrices packed side by side -> one DMA
    LL = nc.dram_tensor("LL", [P, 4 * P], bf16, kind="ExternalInput")
    A = nc.dram_tensor("A", [T - 1, BC], bf16, kind="ExternalOutput")

    # DMA cost is charged to the issuing engine, so balance engine time:
    # SP does nothing else -> most DMA units; Act carries the psum->stage
    # copies -> few units; Pool does DMAs only; DVE (no DMA capability) does
    # saves, pokes, and the copy overflow.  DMAs are [128, 1024] units
    # emitted just-in-time so each engine queue runs in need order.
    dma_pat = pat or (["gpsimd", "sync"] * 5 + ["scalar"]
                      + ["gpsimd", "sync"] * 5 + ["gpsimd", "scalar", "sync"])
    # psum->stage copy engine per (m, ch): mostly Act (it carries few
    # DMAs), a few on DVE; block 0 (the tail) splits so copies run
    # concurrently.
    if copy_dve is None:
        copy_dve = {2, 5, 13, 21, 28, 30}

    def copy_eng(m, ch):
        t = (NB - 1 - m) * NCHN + ch
        return "vector" if t in copy_dve else "scalar"

    with TileContext(nc) as tc:
        with (
            tc.tile_pool(name="cst", bufs=1) as cst,
            tc.tile_pool(name="rp", bufs=4) as rp,
            tc.tile_pool(name="vp", bufs=8) as vp,
            tc.tile_pool(name="stp", bufs=4) as stp,
            tc.tile_pool(name="v0p", bufs=3) as v0p,
            tc.tile_pool(name="ps", bufs=4, space="PSUM") as ps,
        ):
            ll = cst.tile([P, 4 * P], bf16, tag="ll")
            nc.scalar.dma_start(out=ll[:, :], in_=LL[:, :])
            l1 = ll[:, 0:P]
            l1z = ll[:, P:2 * P]
            l3c = ll[:, 2 * P:3 * P]
            l3z = ll[:, 3 * P:4 * P]

            # PE p-state warm-up: ~3.2us of tiny matmuls on a zero tile so
            # the tensor engine is at full clock when the first real matmul
            # issues.  They write the first psum buffer, which the first
            # real matmul resets (start=True), so no cleanup is needed.
            warm = cst.tile([P, 64], bf16, tag="warm")
            nc.gpsimd.memset(warm[:, :], 0.0)
            wpt = ps.tile([P, CH], f32, tag="ps")
            for _ in range(40):
                nc.tensor.matmul(wpt[0:64, 0:64], warm[:, :], warm[:, :],
                                 start=True, stop=True)

            rt, vt = {}, {}
            dma_n = [0]

            def next_q():
                q = dma_pat[dma_n[0] % len(dma_pat)]
                dma_n[0] += 1
                return getattr(nc, q)

            def emit_loads(m):
                r = rp.tile([P, BC], bf16, tag="r")
                v = vp.tile([P, BC], bf16, tag="v")
                for ch in range(NCHN):
                    cs = slice(ch * CH, (ch + 1) * CH)
                    rows = slice(m * P, (m + 1) * P)
                    next_q().dma_start(out=r[:, cs], in_=R[rows, cs])
                    next_q().dma_start(out=v[:, cs], in_=V[rows, cs])
                rt[m] = r
                vt[m] = v

            emit_loads(NB - 1)
            emit_loads(NB - 2)

            v0s = {}

            def save(m):
                kap = (GAMMA / GL) if m == NB - 1 else (GAMMA / GL - 1.0)
                s = v0p.tile([1, BC], bf16, tag="v0")
                nc.vector.tensor_scalar_mul(s[0:1, :], vt[m][0:1, :], kap)
                v0s[m] = s

            save(NB - 1)
            for m in range(NB - 1, -1, -1):
                if m >= 2:
                    emit_loads(m - 2)
                if m >= 1:
                    save(m - 1)
                lhs1 = l1z if m == NB - 1 else l1
                lhs3 = l3z if m == NB - 1 else l3c
                stage = stp.tile([P, BC], bf16, tag="st")
                nrows = P - 1 if m == NB - 1 else P
                for ch in range(NCHN):
                    cs = slice(ch * CH, (ch + 1) * CH)
                    pt = ps.tile([P, CH], f32, tag="ps")
                    for s2 in range(CH // NW):
                        ns = slice(s2 * NW, (s2 + 1) * NW)
                        gs = slice(ch * CH + s2 * NW, ch * CH + (s2 + 1) * NW)
                        nc.tensor.matmul(pt[:, ns], lhs1, rt[m][:, gs],
                                         start=True, stop=False)
                        nc.tensor.matmul(pt[:, ns], lhs3, vt[m][:, gs],
                                         start=False, stop=True)
                    eng = copy_eng(m, ch)
                    if eng == "scalar":
                        nc.scalar.copy(stage[:, cs], pt[:, :])
                    else:
                        nc.vector.tensor_copy(stage[:, cs], pt[:, :])
                    if m >= 1:
                        # carry poke: V_{m-1}[0] = kappa*v0_m + stage_m[0]
                        nc.vector.tensor_tensor(
                            vt[m - 1][0:1, cs], v0s[m][0:1, cs],
                            stage[0:1, cs], add)
                    next_q().dma_start(
                        out=A[m * P:m * P + nrows, cs],
                        in_=stage[0:nrows, cs])
    nc.finalize()
    return nc


_NC_CACHE = None


def kernel(rewards: np.ndarray, values: np.ndarray) -> np.ndarray:
    import ml_dtypes
    from concourse.bass_utils import run_bass_kernel_spmd

    bf = ml_dtypes.bfloat16
    rewards = np.asarray(rewards)
    values = np.asarray(values)

    global _NC_CACHE
    if _NC_CACHE is None:
        _NC_CACHE = _build()
    nc = _NC_CACHE

    LL = _make_consts()
    rbf = rewards.astype(bf)
    vbf = values.astype(bf)
    in_maps = []
    for c in range(NCORES):
        cs = slice(c * BC, (c + 1) * BC)
        in_maps.append({
            "R": np.ascontiguousarray(rbf[:, cs]),
            "V": np.ascontiguousarray(vbf[:, cs]),
            "LL": LL,
        })
    res = run_bass_kernel_spmd(nc, in_maps, core_ids=list(range(NCORES)))
    out = np.empty((T - 1, B), dtype=np.float32)
    for c in range(NCORES):
        out[:, c * BC:(c + 1) * BC] = np.asarray(res.results[c]["A"],
                                                 dtype=np.float32)
    # Repair the carry-displaced -v0 on output rows 0, 128, ..., 768: the
    # device stored psum row 0 = adv + v0 there (L3 row 0 held the carry
    # column instead of its -1 diagonal entry).
    vb32 = vbf.astype(np.float32)
    for m in range(NB - 1):
        out[m * P, :] -= vb32[m * P, :]
    return out
